# revision 4
# baseline (speedup 1.0000x reference)
"""Trainium2 Bass kernel for nn_Conv2d_73967926771856.

The reference computes ``(enc_x @ pad_mat.T) @ weight.T + bias`` where
``weight`` is the dense Toeplitz expansion of a 3x3 conv kernel
(OC=16, IC=8, 32x32 images, pad=1, stride=1) and ``pad_mat`` is the 0/1
zero-padding matrix. Both are exact structural encodings, so the kernel
extracts the 16x8x3x3 conv kernel + per-channel bias from them (pure
element copies, no arithmetic) and runs the conv directly on device:
9 accumulating TensorE matmuls per output tile (one per tap, contraction
over input channels, windowed to handle padding), bias folded in via a
ones-row in the contraction.

Sharding: data-parallel over the batch dim — 8 images per NeuronCore,
outputs concatenated on host.
"""

import re

import numpy as np

import bass_rust
import concourse.bass as bass
import concourse.mybir as mybir
from concourse import tile
from concourse.bass_utils import run_bass_kernel_spmd
from concourse.vector_clock import ScopedClock

# Module config
IC, OC = 8, 16
KH, KW = 3, 3
H, W = 32, 32
PAD = 1
PH, PW = H + 2 * PAD, W + 2 * PAD  # 34x34
OH, OW = 32, 32
B = 64
N_CORES = 8
B_LOC = B // N_CORES  # 8 images per core
HW = H * W  # 1024
N_TAPS = KH * KW  # 9
KC = IC + 1  # contraction rows incl. ones-row for bias


def _patch_tile_drain():
    """This walrus build allows only one sync-wait command per Drain
    (CoreV3 setupSyncWait NO_STRUCT). Split the Tile end-of-kernel drain
    into one drain per ticked logical processor."""
    if getattr(tile.TileContext, "_drain_split_patched", False):
        return

    def _split_drain_and_barrier(self, tick_clock, wait_clock):
        gc = tick_clock.global_clock
        ticks = [int(s) for s in re.findall(r"\d+", repr(gc))]
        for i, t in enumerate(ticks):
            if t > 0:
                v = [0] * len(ticks)
                v[i] = t
                d = self.nc.sync.drain()
                wait_clock.add_sem_waits(
                    d.ins, ScopedClock({None: bass_rust.VectorClock(v)})
                )
        self.nc.all_engine_barrier()
        popped = self.nc._tile_sem_poison_stack.pop()
        assert popped is self._sem_poison
        self.nc.clear_and_free_semaphores(list(self.sems.allocated().values()))
        self.nc.all_engine_barrier()

    tile.TileContext._drain_and_barrier = _split_drain_and_barrier
    tile.TileContext._drain_split_patched = True


def _legalize_sync_waits(nc):
    """This walrus build (CoreV3 setupSyncWait) supports only ONE sync-wait
    command per instruction. Hoist extra waits onto nofuse NoOps inserted
    immediately before the instruction on the same engine."""
    for f in nc.m.functions:
        for bb in f.blocks:
            insts = bb.instructions
            # iterate in reverse so inserts keep earlier indices valid
            for idx in range(len(insts) - 1, -1, -1):
                inst = insts[idx]
                si = inst.sync_info
                if si is None or len(si.on_wait) <= 1:
                    continue
                waits = list(si.on_wait)
                si.on_wait = [waits[-1]]
                for w in reversed(waits[:-1]):
                    nop = mybir.InstNoOp(
                        name=nc.get_next_instruction_name(),
                        sync_info=mybir.SyncInfo(on_wait=[w], on_update=[]),
                        bass_nofuse=True,
                        engine=inst.engine,
                    )
                    nc.register_instruction(nop)
                    insts.insert(idx, nop)


def _tap_window(k, lim=32):
    """Output range [lo, hi) on one axis where input index o+k-1 is in
    bounds."""
    return max(0, 1 - k), min(lim, lim + 1 - k)


def _build_program():
    nc = bass.Bass()
    x_ext = nc.declare_dram_parameter("x", [B_LOC, IC * HW], mybir.dt.float32, isOutput=False)
    wk_ext = nc.declare_dram_parameter("wk", [KC, N_TAPS * OC], mybir.dt.float32, isOutput=False)
    out_ext = nc.declare_dram_parameter("out", [B_LOC, OC * OH * OW], mybir.dt.float32, isOutput=True)

    f32 = mybir.dt.float32
    with tile.TileContext(nc) as tc:
        with (
            tc.tile_pool(name="xp", bufs=1) as xp,
            tc.tile_pool(name="wp", bufs=1) as wp,
            tc.tile_pool(name="op", bufs=4) as op,
            tc.tile_pool(name="ps", bufs=8, space="PSUM") as ps,
        ):
            # x in SBUF as [ic (9 partitions incl ones-row), b, h*w]
            xs = xp.tile([KC, B_LOC, HW], f32)
            wt = wp.tile([KC, N_TAPS * OC], f32)
            nc.sync.dma_start(wt[:], wk_ext[:])
            # ones everywhere, then the 8 image channels DMA'd over rows 0..7;
            # row 8 stays at 1.0 (the bias ones-row).
            nc.vector.memset(xs[:], 1.0)
            nc.sync.dma_start(
                xs[0:IC, :, :], x_ext[:].rearrange("b (c f) -> c b f", c=IC)
            )

            # taps ordered center-first (full window -> start=True)
            taps = [(1, 1)] + [
                (kh, kw) for kh in range(KH) for kw in range(KW) if (kh, kw) != (1, 1)
            ]
            for b in range(B_LOC):
                for h in range(2):  # halves of the 32 output rows
                    pt = ps.tile([OC, 512], f32)
                    pt3 = pt[:].rearrange("p (oh ow) -> p oh ow", ow=OW)
                    xs3 = xs[:, b, :].rearrange("p (r c) -> p r c", c=W)
                    for ti, (kh, kw) in enumerate(taps):
                        t = kh * KW + kw
                        oh_lo, oh_hi = _tap_window(kh)
                        ow_lo, ow_hi = _tap_window(kw)
                        # clamp to this tile's oh range [16h, 16h+16)
                        oh_lo = max(oh_lo, 16 * h)
                        oh_hi = min(oh_hi, 16 * h + 16)
                        kc = KC if (kh, kw) == (1, 1) else IC
                        lhsT = wt[0:kc, t * OC : (t + 1) * OC]
                        rhs = xs3[0:kc, oh_lo + kh - 1 : oh_hi + kh - 1, ow_lo + kw - 1 : ow_hi + kw - 1]
                        dst = pt3[:, oh_lo - 16 * h : oh_hi - 16 * h, ow_lo:ow_hi]
                        nc.tensor.matmul(
                            dst,
                            lhsT,
                            rhs,
                            start=(ti == 0),
                            stop=(ti == len(taps) - 1),
                            skip_group_check=True,
                        )
                    ot = op.tile([OC, 512], f32)
                    if (b + h) % 2 == 0:
                        nc.vector.tensor_copy(ot[:], pt[:])
                    else:
                        nc.scalar.copy(ot[:], pt[:])
                    nc.sync.dma_start(
                        out_ext[:].rearrange("b (oc f) -> b oc f", oc=OC)[
                            b, :, 512 * h : 512 * (h + 1)
                        ],
                        ot[:],
                    )
    _legalize_sync_waits(nc)
    return nc


def _extract_weights(weight, bias):
    """Exact extraction of the conv kernel + per-channel bias from the
    Toeplitz matrix: weight[(oc*OH+oh)*OW+ow, (ic*PH+oh+kh)*PW+(ow+kw)]
    == k3[oc, ic, kh, kw] for every valid row; row (oh,ow)=(0,0) is used.
    Packs lhsT tap blocks [KC, tap*OC+oc]: row ic -> k3[oc,ic,kh,kw],
    row IC -> bias (center tap only, multiplied by a ones-row in rhs)."""
    w = np.asarray(weight, dtype=np.float32)
    bvec = np.asarray(bias, dtype=np.float32)
    wk = np.zeros((KC, N_TAPS * OC), dtype=np.float32)
    for kh in range(KH):
        for kw in range(KW):
            t = kh * KW + kw
            for ic in range(IC):
                wk[ic, t * OC : (t + 1) * OC] = w[
                    np.arange(OC) * OH * OW, ic * PH * PW + kh * PW + kw
                ]
    wk[IC, 4 * OC : 5 * OC] = bvec[np.arange(OC) * OH * OW]
    return wk


_cached = {}


def kernel(enc_x, weight, bias, pad_mat):
    _patch_tile_drain()
    x = np.ascontiguousarray(np.asarray(enc_x, dtype=np.float32))
    wk = _extract_weights(weight, bias)

    if "nc" not in _cached:
        _cached["nc"] = _build_program()
    nc = _cached["nc"]

    in_maps = [
        {"x": x[i * B_LOC : (i + 1) * B_LOC], "wk": wk} for i in range(N_CORES)
    ]
    res = run_bass_kernel_spmd(nc, in_maps, core_ids=list(range(N_CORES)))
    out = np.concatenate([res.results[i]["out"] for i in range(N_CORES)], axis=0)
    return out.astype(np.float32)


# revision 12
# speedup vs baseline: 6.5218x; 6.5218x over previous
"""Trainium2 Bass kernel for nn_Conv2d_73967926771856.

The reference computes ``(enc_x @ pad_mat.T) @ weight.T + bias`` where
``weight`` is the dense Toeplitz expansion of a 3x3 conv kernel
(OC=16, IC=8, 32x32 images, pad=1, stride=1) and ``pad_mat`` is the 0/1
zero-padding matrix. Both are exact structural encodings, so the kernel
extracts the 16x8x3x3 conv kernel + per-channel bias from them (pure
element copies, no arithmetic) and runs the conv directly on device.

Device algorithm (per core, batch-sharded 8 images/core):
 - inputs are split into bf16 (hi, lo) pairs on host: x = hi + lo with
   hi = bf16(x); full-speed bf16 matmuls then compute
   x*w ~= xh*wh + xl*wh + xh*wl (the xl*wl term is ~2^-18 relative).
 - the 3x3 conv contracts (ic, kh) pairs in the matmul contraction dim:
   3 kh-shifted copies of each channel stacked on partitions
   (3 hi/lo blocks x 3 shifts x 8 channels = 72 rows), kw becomes 3
   accumulating matmul taps whose column-windowed access patterns
   realize the horizontal zero padding. Vertical padding rows are
   zeroed host-side; the whole 72-row operand is assembled on host so
   each image is a single dense DMA.
 - per-channel bias is added during PSUM->SBUF eviction (ACT bias /
   DVE tensor-scalar add), alternating between both engines.

Sharding: data-parallel over the batch dim; host concatenates the
per-core [8, 16384] outputs.
"""

import re

import ml_dtypes
import numpy as np

import bass_rust
import concourse.bass as bass
import concourse.mybir as mybir
from concourse import tile
from concourse.bass_utils import run_bass_kernel_spmd
from concourse.vector_clock import ScopedClock

# Module config
IC, OC = 8, 16
KH, KW = 3, 3
H, W = 32, 32
PAD = 1
PH, PW = H + 2 * PAD, W + 2 * PAD  # 34x34
OH, OW = 32, 32
B = 64
N_CORES = 8
B_LOC = B // N_CORES  # 8 images per core
HW = H * W  # 1024
KROWS = 3 * KH * IC  # 72 contraction rows: (hi, lo, hi-dup) x (kh, ic)

BF16 = ml_dtypes.bfloat16


def _patch_tile_drain():
    """Replace Tile's end-of-kernel drain + double EVSEM barrier + sem
    reset (~9us) with a minimal gather-and-reset (~1us):
      - every non-gpsimd engine bumps a 'done' sem as its last
        instruction (after its final sem wait, so no reset race);
      - gpsimd waits for those bumps plus every logical processor's
        final tick (DMA queues included), then resets DMA queues and
        semaphores.
    The framework's entry barrier at the top of the NEFF orders the
    reset against any subsequent execution. Also works around this
    walrus build's one-sync-wait-per-instruction limit by emitting one
    waiting NoOp per processor."""
    if getattr(tile.TileContext, "_drain_split_patched", False):
        return

    import os

    fast = os.environ.get("BASS_FAST_CLEANUP", "0") == "1"

    def _per_proc_waits(self, tick_clock, wait_clock, engine_builder):
        gc = tick_clock.global_clock
        ticks = [int(s) for s in re.findall(r"\d+", repr(gc))]
        for i, t in enumerate(ticks):
            if t > 0:
                v = [0] * len(ticks)
                v[i] = t
                n = engine_builder.nop(nofuse=True)
                wait_clock.add_sem_waits(
                    n.ins, ScopedClock({None: bass_rust.VectorClock(v)})
                )

    def _fast_drain_and_barrier(self, tick_clock, wait_clock):
        nc = self.nc
        done = nc.alloc_semaphore(f"tile_done_{nc.next_id()}")
        n_done = 0
        for eng_type, eng in nc.engines.items():
            if eng_type == mybir.EngineType.Pool:
                continue
            eng.nop(nofuse=True).then_inc(done, 1)
            n_done += 1
        nc.gpsimd.wait_ge(done, n_done)
        _per_proc_waits(self, tick_clock, wait_clock, nc.gpsimd)
        popped = nc._tile_sem_poison_stack.pop()
        assert popped is self._sem_poison
        nc.clear_and_free_semaphores(
            list(self.sems.allocated().values()) + [done]
        )

    def _safe_drain_and_barrier(self, tick_clock, wait_clock):
        nc = self.nc
        _per_proc_waits(self, tick_clock, wait_clock, nc.sync)
        nc.sync.drain()
        nc.all_engine_barrier()
        popped = nc._tile_sem_poison_stack.pop()
        assert popped is self._sem_poison
        nc.clear_and_free_semaphores(list(self.sems.allocated().values()))
        nc.all_engine_barrier()

    tile.TileContext._drain_and_barrier = (
        _fast_drain_and_barrier if fast else _safe_drain_and_barrier
    )
    tile.TileContext._drain_split_patched = True


def _legalize_sync_waits(nc):
    """Hoist extra sync waits (>1 per instruction) onto nofuse NoOps
    inserted immediately before the instruction on the same engine."""
    for f in nc.m.functions:
        for bb in f.blocks:
            insts = bb.instructions
            for idx in range(len(insts) - 1, -1, -1):
                inst = insts[idx]
                si = inst.sync_info
                if si is None or len(si.on_wait) <= 1:
                    continue
                waits = list(si.on_wait)
                si.on_wait = [waits[-1]]
                for w in reversed(waits[:-1]):
                    nop = mybir.InstNoOp(
                        name=nc.get_next_instruction_name(),
                        sync_info=mybir.SyncInfo(on_wait=[w], on_update=[]),
                        bass_nofuse=True,
                        engine=inst.engine,
                    )
                    nc.register_instruction(nop)
                    insts.insert(idx, nop)


def _build_program():
    nc = bass.Bass()
    f32 = mybir.dt.float32
    bf16 = mybir.dt.bfloat16
    # xs content is assembled host-side: [b][72 rows][32x32] bf16
    xs_ext = nc.declare_dram_parameter("xs", [B_LOC, KROWS, H * W], bf16, isOutput=False)
    wk_ext = nc.declare_dram_parameter("wk", [KROWS, KW * OC], bf16, isOutput=False)
    bias_ext = nc.declare_dram_parameter("bias", [OC, 1], f32, isOutput=False)
    out_ext = nc.declare_dram_parameter("out", [B_LOC, OC * OH * OW], f32, isOutput=True)

    dma_engines = [nc.sync, nc.gpsimd, nc.scalar]

    with tile.TileContext(nc) as tc:
        with (
            tc.tile_pool(name="xp", bufs=1) as xp,
            tc.tile_pool(name="wp", bufs=1) as wp,
            tc.tile_pool(name="op", bufs=4) as op,
            tc.tile_pool(name="ps", bufs=8, space="PSUM") as ps,
        ):
            wt = wp.tile([KROWS, KW * OC], bf16)
            bt = wp.tile([OC, 1], f32)
            nc.sync.dma_start(wt[:], wk_ext[:])
            nc.sync.dma_start(bt[:], bias_ext[:])

            xtiles = []
            for b in range(B_LOC):
                xt = xp.tile([KROWS, H, W], bf16, tag=f"xs{b}")
                dma_engines[b % 3].dma_start(
                    xt[:], xs_ext[b].rearrange("k (r c) -> k r c", c=W)
                )
                xtiles.append(xt)

            for b in range(B_LOC):
                ot = op.tile([OC, 2 * 512], f32, tag="ot")
                for h in range(2):  # halves of the 32 output rows
                    pt = ps.tile([OC, 512], f32)
                    pt3 = pt[:].rearrange("p (oh ow) -> p oh ow", ow=OW)
                    # taps: kw=1 (full window, starts accumulation), then 0, 2
                    for ti, kw in enumerate((1, 0, 2)):
                        ow_lo, ow_hi = max(0, 1 - kw), min(OW, OW + 1 - kw)
                        nc.tensor.matmul(
                            pt3[:, :, ow_lo:ow_hi],
                            wt[:, kw * OC : (kw + 1) * OC],
                            xtiles[b][
                                :,
                                16 * h : 16 * h + 16,
                                ow_lo + kw - 1 : ow_hi + kw - 1,
                            ],
                            start=(ti == 0),
                            stop=(ti == 2),
                            skip_group_check=True,
                        )
                    if h == 0:
                        nc.scalar.activation(
                            ot[:, :512], pt[:],
                            mybir.ActivationFunctionType.Identity, bias=bt[:],
                        )
                    else:
                        nc.vector.tensor_scalar_add(ot[:, 512:], pt[:], bt[:])
                nc.sync.dma_start(
                    out_ext[:].rearrange("b (oc f) -> b oc f", oc=OC)[b], ot[:]
                )
    _legalize_sync_waits(nc)
    return nc


def _extract_weights(weight, bias):
    """Exact extraction of the conv kernel + per-channel bias from the
    Toeplitz matrix: weight[(oc*OH+oh)*OW+ow, (ic*PH+oh+kh)*PW+(ow+kw)]
    == k3[oc, ic, kh, kw] for every valid row; row (oh,ow)=(0,0) is
    used. Returns (wk_bf16 [KROWS, KW*OC], bias [OC,1] f32)."""
    w = np.asarray(weight, dtype=np.float32)
    bvec = np.asarray(bias, dtype=np.float32)[np.arange(OC) * OH * OW]
    k3 = np.empty((OC, IC, KH, KW), dtype=np.float32)
    for kh in range(KH):
        for kw in range(KW):
            for ic in range(IC):
                k3[:, ic, kh, kw] = w[
                    np.arange(OC) * OH * OW, ic * PH * PW + kh * PW + kw
                ]
    k_hi = k3.astype(BF16)
    k_lo = (k3 - k_hi.astype(np.float32)).astype(BF16)
    wk = np.zeros((KROWS, KW * OC), dtype=BF16)
    for kw in range(KW):
        for kh in range(KH):
            for ic in range(IC):
                r = kh * IC + ic
                wk[0 * KH * IC + r, kw * OC : (kw + 1) * OC] = k_hi[:, ic, kh, kw]
                wk[1 * KH * IC + r, kw * OC : (kw + 1) * OC] = k_hi[:, ic, kh, kw]
                wk[2 * KH * IC + r, kw * OC : (kw + 1) * OC] = k_lo[:, ic, kh, kw]
    return wk, bvec.reshape(OC, 1)


def _prep_x(enc_x):
    """Host prep: split into bf16 hi/lo pairs and assemble the 72-row
    matmul operand per image: rows (block, kh, ic) hold the image
    shifted vertically by kh-1 with zero padding, blocks = (hi, lo, hi).
    Returns [B, KROWS, H*W] bf16."""
    x = np.asarray(enc_x, dtype=np.float32).reshape(B, IC, H, W)
    xh = x.astype(BF16)
    xl = (x - xh.astype(np.float32)).astype(BF16)
    out = np.zeros((B, 3, KH, IC, H, W), dtype=BF16)
    for blk, a in enumerate((xh, xl, xh)):
        for kh in range(KH):
            s = kh - 1  # source row = r + s
            r_lo, r_hi = max(0, -s), min(H, H - s)
            out[:, blk, kh, :, r_lo:r_hi, :] = a[
                :, :, r_lo + s : r_hi + s, :
            ].transpose(0, 1, 2, 3)
    return np.ascontiguousarray(out.reshape(B, KROWS, H * W))


_cached = {}


def _make_in_maps(enc_x, weight, bias):
    xs = _prep_x(enc_x)
    wk, bvec = _extract_weights(weight, bias)
    return [
        {
            "xs": xs[i * B_LOC : (i + 1) * B_LOC],
            "wk": wk,
            "bias": bvec,
        }
        for i in range(N_CORES)
    ]


def kernel(enc_x, weight, bias, pad_mat):
    _patch_tile_drain()
    in_maps = _make_in_maps(enc_x, weight, bias)

    if "nc" not in _cached:
        _cached["nc"] = _build_program()
    nc = _cached["nc"]

    res = run_bass_kernel_spmd(nc, in_maps, core_ids=list(range(N_CORES)))
    out = np.concatenate([res.results[i]["out"] for i in range(N_CORES)], axis=0)
    return out.astype(np.float32)


# revision 14
# speedup vs baseline: 9.4369x; 1.4470x over previous
"""Trainium2 Bass kernel for nn_Conv2d_73967926771856.

The reference computes ``(enc_x @ pad_mat.T) @ weight.T + bias`` where
``weight`` is the dense Toeplitz expansion of a 3x3 conv kernel
(OC=16, IC=8, 32x32 images, pad=1, stride=1) and ``pad_mat`` is the 0/1
zero-padding matrix. Both are exact structural encodings, so the kernel
extracts the 16x8x3x3 conv kernel + per-channel bias from them (pure
element copies, no arithmetic) and runs the conv directly on device.

Device algorithm (per core, batch-sharded 8 images/core):
 - inputs are split into bf16 (hi, lo) pairs on host: x = hi + lo with
   hi = bf16(x); full-speed bf16 matmuls then compute
   x*w ~= xh*wh + xl*wh + xh*wl (the xl*wl term is ~2^-18 relative).
 - the 3x3 conv contracts (ic, kh) pairs in the matmul contraction dim:
   3 kh-shifted copies of each channel stacked on partitions
   (3 hi/lo blocks x 3 shifts x 8 channels = 72 rows), kw becomes 3
   accumulating matmul taps whose column-windowed access patterns
   realize the horizontal zero padding. Vertical padding rows are
   zeroed host-side; the whole 72-row operand is assembled on host so
   each image is a single dense DMA.
 - per-channel bias is added during PSUM->SBUF eviction (ACT bias /
   DVE tensor-scalar add), alternating between both engines.

Sharding: data-parallel over the batch dim; host concatenates the
per-core [8, 16384] outputs.
"""

import re

import ml_dtypes
import numpy as np

import bass_rust
import concourse.bass as bass
import concourse.mybir as mybir
from concourse import tile
from concourse.bass_utils import run_bass_kernel_spmd
from concourse.vector_clock import ScopedClock

# Module config
IC, OC = 8, 16
KH, KW = 3, 3
H, W = 32, 32
PAD = 1
PH, PW = H + 2 * PAD, W + 2 * PAD  # 34x34
OH, OW = 32, 32
B = 64
N_CORES = 8
B_LOC = B // N_CORES  # 8 images per core
HW = H * W  # 1024
KROWS = 3 * KH * IC  # 72 contraction rows: (hi, lo, hi-dup) x (kh, ic)

BF16 = ml_dtypes.bfloat16


def _patch_tile_drain():
    """Replace Tile's end-of-kernel drain + double EVSEM barrier + sem
    reset (~9us) with a minimal gather-and-reset (~1us):
      - every non-gpsimd engine bumps a 'done' sem as its last
        instruction (after its final sem wait, so no reset race);
      - gpsimd waits for those bumps plus every logical processor's
        final tick (DMA queues included), then resets DMA queues and
        semaphores.
    The framework's entry barrier at the top of the NEFF orders the
    reset against any subsequent execution. Also works around this
    walrus build's one-sync-wait-per-instruction limit by emitting one
    waiting NoOp per processor."""
    if getattr(tile.TileContext, "_drain_split_patched", False):
        return

    import os

    fast = os.environ.get("BASS_FAST_CLEANUP", "0") == "1"

    def _per_proc_waits(self, tick_clock, wait_clock, engine_builder):
        gc = tick_clock.global_clock
        ticks = [int(s) for s in re.findall(r"\d+", repr(gc))]
        for i, t in enumerate(ticks):
            if t > 0:
                v = [0] * len(ticks)
                v[i] = t
                n = engine_builder.nop(nofuse=True)
                wait_clock.add_sem_waits(
                    n.ins, ScopedClock({None: bass_rust.VectorClock(v)})
                )

    def _fast_drain_and_barrier(self, tick_clock, wait_clock):
        nc = self.nc
        done = nc.alloc_semaphore(f"tile_done_{nc.next_id()}")
        n_done = 0
        for eng_type, eng in nc.engines.items():
            if eng_type == mybir.EngineType.Pool:
                continue
            eng.nop(nofuse=True).then_inc(done, 1)
            n_done += 1
        nc.gpsimd.wait_ge(done, n_done)
        _per_proc_waits(self, tick_clock, wait_clock, nc.gpsimd)
        popped = nc._tile_sem_poison_stack.pop()
        assert popped is self._sem_poison
        nc.clear_and_free_semaphores(
            list(self.sems.allocated().values()) + [done]
        )

    def _safe_drain_and_barrier(self, tick_clock, wait_clock):
        nc = self.nc
        _per_proc_waits(self, tick_clock, wait_clock, nc.sync)
        nc.sync.drain()
        nc.all_engine_barrier()
        popped = nc._tile_sem_poison_stack.pop()
        assert popped is self._sem_poison
        nc.clear_and_free_semaphores(list(self.sems.allocated().values()))
        nc.all_engine_barrier()

    tile.TileContext._drain_and_barrier = (
        _fast_drain_and_barrier if fast else _safe_drain_and_barrier
    )
    tile.TileContext._drain_split_patched = True


def _legalize_sync_waits(nc):
    """Hoist extra sync waits (>1 per instruction) onto nofuse NoOps
    inserted immediately before the instruction on the same engine."""
    for f in nc.m.functions:
        for bb in f.blocks:
            insts = bb.instructions
            for idx in range(len(insts) - 1, -1, -1):
                inst = insts[idx]
                si = inst.sync_info
                if si is None or len(si.on_wait) <= 1:
                    continue
                waits = list(si.on_wait)
                si.on_wait = [waits[-1]]
                for w in reversed(waits[:-1]):
                    nop = mybir.InstNoOp(
                        name=nc.get_next_instruction_name(),
                        sync_info=mybir.SyncInfo(on_wait=[w], on_update=[]),
                        bass_nofuse=True,
                        engine=inst.engine,
                    )
                    nc.register_instruction(nop)
                    insts.insert(idx, nop)


def _build_program():
    nc = bass.Bass()
    f32 = mybir.dt.float32
    bf16 = mybir.dt.bfloat16
    # xs content is assembled host-side: [b][72 rows][32x32] bf16
    xs_ext = nc.declare_dram_parameter("xs", [B_LOC, KROWS, H * W], bf16, isOutput=False)
    wk_ext = nc.declare_dram_parameter("wk", [KROWS, KW * OC], bf16, isOutput=False)
    bias_ext = nc.declare_dram_parameter("bias", [128, 1], f32, isOutput=False)
    out_ext = nc.declare_dram_parameter("out", [B_LOC, OC * OH * OW], f32, isOutput=True)

    dma_engines = [nc.sync, nc.gpsimd, nc.scalar]
    out3 = out_ext[:].rearrange("b (oc f) -> b oc f", oc=OC)

    with tile.TileContext(nc) as tc:
        with (
            tc.tile_pool(name="xp", bufs=1) as xp,
            tc.tile_pool(name="wp", bufs=1) as wp,
            tc.tile_pool(name="op", bufs=4) as op,
            tc.tile_pool(name="ps", bufs=8, space="PSUM") as ps,
        ):
            wt = wp.tile([KROWS, KW * OC], bf16)
            bt = wp.tile([128, 1], f32)
            nc.sync.dma_start(wt[:], wk_ext[:])
            nc.sync.dma_start(bt[:], bias_ext[:])

            xtiles = []
            for b in range(B_LOC):
                xt = xp.tile([KROWS, H, W], bf16, tag=f"xs{b}")
                dma_engines[b % 3].dma_start(
                    xt[:], xs_ext[b].rearrange("k (r c) -> k r c", c=W)
                )
                xtiles.append(xt)

            # 4 rounds; round r computes tiles t=4r..4r+3 (t=(b,h)) in the
            # 4 PE column groups concurrently, into one PSUM bank.
            for r in range(4):
                pt = ps.tile([128, 512], f32)
                # taps: kw=1 (full window, starts accumulation), then 0, 2
                for ti, kw in enumerate((1, 0, 2)):
                    ow_lo, ow_hi = max(0, 1 - kw), min(OW, OW + 1 - kw)
                    for j in range(4):
                        t = 4 * r + j
                        b, h = t // 2, t % 2
                        dst = pt[32 * j : 32 * j + OC, :].rearrange(
                            "p (oh ow) -> p oh ow", ow=OW
                        )[:, :, ow_lo:ow_hi]
                        nc.tensor.matmul(
                            dst,
                            wt[:, kw * OC : (kw + 1) * OC],
                            xtiles[b][
                                :,
                                16 * h : 16 * h + 16,
                                ow_lo + kw - 1 : ow_hi + kw - 1,
                            ],
                            start=(ti == 0),
                            stop=(ti == 2),
                            skip_group_check=True,
                            tile_position=(0, 32 * j),
                        )
                ot = op.tile([128, 512], f32, tag="ot")
                if r % 2 == 0:
                    nc.scalar.activation(
                        ot[:], pt[:],
                        mybir.ActivationFunctionType.Identity, bias=bt[:],
                    )
                else:
                    nc.vector.tensor_scalar_add(ot[:], pt[:], bt[:])
                for j in range(4):
                    t = 4 * r + j
                    b, h = t // 2, t % 2
                    (nc.sync if j % 2 == 0 else nc.scalar).dma_start(
                        out3[b, :, 512 * h : 512 * (h + 1)],
                        ot[32 * j : 32 * j + OC, :],
                    )
    _legalize_sync_waits(nc)
    return nc


def _extract_weights(weight, bias):
    """Exact extraction of the conv kernel + per-channel bias from the
    Toeplitz matrix: weight[(oc*OH+oh)*OW+ow, (ic*PH+oh+kh)*PW+(ow+kw)]
    == k3[oc, ic, kh, kw] for every valid row; row (oh,ow)=(0,0) is
    used. Returns (wk_bf16 [KROWS, KW*OC], bias [OC,1] f32)."""
    w = np.asarray(weight, dtype=np.float32)
    bvec = np.asarray(bias, dtype=np.float32)[np.arange(OC) * OH * OW]
    k3 = np.empty((OC, IC, KH, KW), dtype=np.float32)
    for kh in range(KH):
        for kw in range(KW):
            for ic in range(IC):
                k3[:, ic, kh, kw] = w[
                    np.arange(OC) * OH * OW, ic * PH * PW + kh * PW + kw
                ]
    k_hi = k3.astype(BF16)
    k_lo = (k3 - k_hi.astype(np.float32)).astype(BF16)
    wk = np.zeros((KROWS, KW * OC), dtype=BF16)
    for kw in range(KW):
        for kh in range(KH):
            for ic in range(IC):
                r = kh * IC + ic
                wk[0 * KH * IC + r, kw * OC : (kw + 1) * OC] = k_hi[:, ic, kh, kw]
                wk[1 * KH * IC + r, kw * OC : (kw + 1) * OC] = k_hi[:, ic, kh, kw]
                wk[2 * KH * IC + r, kw * OC : (kw + 1) * OC] = k_lo[:, ic, kh, kw]
    # bias replicated into each of the 4 PE column-group partition strips
    bias_full = np.zeros((128, 1), dtype=np.float32)
    for j in range(4):
        bias_full[32 * j : 32 * j + OC, 0] = bvec
    return wk, bias_full


def _prep_x(enc_x):
    """Host prep: split into bf16 hi/lo pairs and assemble the 72-row
    matmul operand per image: rows (block, kh, ic) hold the image
    shifted vertically by kh-1 with zero padding, blocks = (hi, lo, hi).
    Returns [B, KROWS, H*W] bf16."""
    x = np.asarray(enc_x, dtype=np.float32).reshape(B, IC, H, W)
    xh = x.astype(BF16)
    xl = (x - xh.astype(np.float32)).astype(BF16)
    out = np.zeros((B, 3, KH, IC, H, W), dtype=BF16)
    for blk, a in enumerate((xh, xl, xh)):
        for kh in range(KH):
            s = kh - 1  # source row = r + s
            r_lo, r_hi = max(0, -s), min(H, H - s)
            out[:, blk, kh, :, r_lo:r_hi, :] = a[
                :, :, r_lo + s : r_hi + s, :
            ].transpose(0, 1, 2, 3)
    return np.ascontiguousarray(out.reshape(B, KROWS, H * W))


_cached = {}


def _make_in_maps(enc_x, weight, bias):
    xs = _prep_x(enc_x)
    wk, bvec = _extract_weights(weight, bias)
    return [
        {
            "xs": xs[i * B_LOC : (i + 1) * B_LOC],
            "wk": wk,
            "bias": bvec,
        }
        for i in range(N_CORES)
    ]


def kernel(enc_x, weight, bias, pad_mat):
    _patch_tile_drain()
    in_maps = _make_in_maps(enc_x, weight, bias)

    if "nc" not in _cached:
        _cached["nc"] = _build_program()
    nc = _cached["nc"]

    res = run_bass_kernel_spmd(nc, in_maps, core_ids=list(range(N_CORES)))
    out = np.concatenate([res.results[i]["out"] for i in range(N_CORES)], axis=0)
    return out.astype(np.float32)
